# revision 22
# baseline (speedup 1.0000x reference)
"""Trainium2 Bass kernel for nn_Atoms (8 NeuronCores, batch-parallel).

Per (b,e) pair: rfft_N -> shape mult -> irfft_N -> gaussian envelope mult ->
zero-padded rfft_2N -> phase mult -> irfft_2N[:N] -> windowed frame DFT ->
resonance scan (tensor_tensor_scan) -> inverse frame DFT -> overlap-add ->
event sum -> max_norm.  All FFTs are 4-step matmul FFTs (P=128 x Q free).
See algo.py for the validated numpy model of the same structure.
"""
import sys
sys.path.insert(0, '/opt/trn_rl_repo')
import numpy as np

P = 128
NS = 32768
Q1 = 256
Q2 = 512
NCB = 16385
WIN = 512
NCO = 257
NF = 128
CT = [(0, 86), (86, 172), (172, 257)]
DEBUG = None


def _wm(n, m, denom, sign, scale=1.0):
    return np.exp(sign * 2j * np.pi * np.outer(np.arange(n), np.arange(m)) / denom) * scale


def _chunk(a, rows=128):
    """[R, C] -> [nch, rows, C] zero-padded."""
    R, C = a.shape
    nch = (R + rows - 1) // rows
    out = np.zeros((nch, rows, C), a.dtype)
    for i in range(nch):
        out[i, :min(rows, R - i * rows), :] = a[i * rows:(i + 1) * rows, :]
    return out


def build_consts():
    c = {}
    s, si = -1, +1
    WPf = _wm(P, P, P, s, 1.0 / np.sqrt(NS))
    c['wpf_r'], c['wpf_i'] = WPf.real, WPf.imag
    Twf = _wm(P, Q1, NS, s)
    c['twf_r'], c['twf_i'] = Twf.real, Twf.imag
    WQf = _wm(Q1, Q1, Q1, s)                       # [n2, k2]
    c['wqf_r'], c['wqf_i'] = _chunk(WQf.real), _chunk(WQf.imag)   # [2,128,256]
    WPi = _wm(P, P, P, si, 1.0 / np.sqrt(NS))
    c['wpi_r'], c['wpi_i'] = WPi.real, WPi.imag
    Twi = _wm(P, Q1, NS, si)
    c['twi_r'], c['twi_i'] = Twi.real, Twi.imag
    WQi = _wm(Q1, Q1, Q1, si)
    c['wqi_r'], c['wqi_i'] = _chunk(WQi.real), _chunk(WQi.imag)
    WPf2 = _wm(P, P, P, s, 1.0 / np.sqrt(2 * NS))
    c['wpf2_r'], c['wpf2_i'] = WPf2.real, WPf2.imag
    Twf2 = _wm(P, Q2, 2 * NS, s)
    c['twf2_r'], c['twf2_i'] = Twf2.real, Twf2.imag
    WQf2 = _wm(Q2, NCO, Q2, s)                     # [n2, k2<=256] 512x257
    c['wqf2_r'], c['wqf2_i'] = _chunk(WQf2.real), _chunk(WQf2.imag)  # [4,128,257]
    WPi2 = _wm(P, P, P, si, 1.0 / np.sqrt(2 * NS))
    c['wpi2_r'], c['wpi2_i'] = WPi2.real, WPi2.imag
    Twi2 = _wm(P, Q2, 2 * NS, si)
    c['twi2_r'], c['twi2_i'] = Twi2.real, Twi2.imag
    c['wpi2h_r'], c['wpi2h_i'] = WPi2.real[64:128, :].copy(), WPi2.imag[64:128, :].copy()
    WQi2 = _wm(Q2, Q1, Q2, si)                     # [s2, t2<256] 512x256
    c['wqi2_r'], c['wqi2_i'] = _chunk(WQi2.real), _chunk(WQi2.imag)  # [4,128,256]
    w = np.arange(WIN)
    ham = 0.54 - 0.46 * np.cos(2.0 * np.pi * w / WIN)
    D = np.exp(-2j * np.pi * np.outer(w, np.arange(NCO)) / WIN) / np.sqrt(WIN)
    hamD = ham[:, None] * D                        # [512, 257]
    c['hd_r'], c['hd_i'] = _chunk(hamD.real), _chunk(hamD.imag)      # [4,128,257]
    coef = np.ones(NCO); coef[1:256] = 2.0
    ang = 2.0 * np.pi * np.outer(np.arange(NCO), np.arange(WIN)) / WIN
    Er = (coef[:, None] * np.cos(ang)) / np.sqrt(WIN)    # [257, 512]
    Ei = (-(coef[:, None]) * np.sin(ang)) / np.sqrt(WIN)
    c['e_r'], c['e_i'] = _chunk(Er, 86), _chunk(Ei, 86)  # [3, 86, 512]
    t = np.arange(P)[:, None] + 128.0 * np.arange(Q1)[None, :]
    c['tsq'] = t * t
    c['ident'] = np.eye(P)
    c['nident'] = -np.eye(P)
    c['jrev'] = np.eye(P)[::-1].copy()
    c['njrev'] = -np.eye(P)[::-1].copy()
    c['ones1'] = np.ones((1, P))
    return {k: np.ascontiguousarray(v, dtype=np.float32) for k, v in c.items()}


def build_pair_data(x, noise):
    B, E = x.shape[:2]
    x = np.clip(x.astype(np.float64), 0.0, 1.0)
    means = x[..., 0]; stds = x[..., 1]
    res = 0.01 + 0.99 * x[..., 2:259]
    spec_shape = x[..., 259:-1]
    amps = x[..., -1]
    sigma = np.clip((1e-8 + stds) * NS, 0.0, NS - 1.0)
    d = {}
    pos = np.clip((np.arange(NCB) + 0.5) * (128.0 / NCB) - 0.5, 0.0, 127.0)
    i0 = np.floor(pos).astype(int); i1 = np.minimum(i0 + 1, 127); wgt = pos - i0
    shp = spec_shape[..., i0] * (1.0 - wgt) + spec_shape[..., i1] * wgt
    full = np.zeros((B, E, NS))
    full[..., :NCB] = shp
    full[..., NCB:] = shp[..., 1:NCB - 1][..., ::-1]
    d['ginv'] = full.reshape(B, E, P, Q1)
    c1 = -0.5 / (sigma * sigma)
    corr = 1.0 / (1.0 + 1e-8 * sigma * np.sqrt(2.0 * np.pi))
    p3 = np.stack([np.repeat(c1[..., None], P, -1),
                   np.repeat(np.log(corr)[..., None], P, -1),
                   np.repeat(amps[..., None], P, -1)], axis=-1)   # [B,E,128,3]
    d['p3'] = p3
    theta = 2.0 * np.pi * (means * 32768.0) / 32769.0
    u = np.exp(-1j * theta[..., None] * np.arange(P))
    v = np.exp(-1j * theta[..., None] * 128.0 * np.arange(NCO))
    uv = np.concatenate([
        np.stack([u.real, u.imag], axis=-2),                      # [B,E,2,128]
        np.stack([v.real, -v.imag], axis=-2),                     # [B,E,2,257]
        np.stack([v.imag, v.real], axis=-2)], axis=-1)            # -> [B,E,2,642]
    d['uv'] = uv
    r3 = np.zeros(res.shape[:-1] + (86, 3))
    r3[..., 0:86, 0] = res[..., 0:86]
    r3[..., 0:86, 1] = res[..., 86:172]
    r3[..., 0:85, 2] = res[..., 172:257]
    d['res3'] = r3
    d['noise'] = noise
    return {k: np.ascontiguousarray(v, dtype=np.float32) for k, v in d.items()}


def build_program(n_batch, n_event):
    import concourse.bass as bass
    import concourse.mybir as mybir
    from concourse.tile import TileContext as TileContextSplitDrain

    def split_excess_waits(nc_, max_waits=1):
        # this container's walrus rejects instructions with >2 sync waits;
        # hoist excess waits onto same-engine NoOps inserted before them.
        n_split = 0
        for f in nc_.m.functions:
            for bb in f.blocks:
                out = []
                for inst in bb.instructions:
                    si = inst.sync_info
                    waits = list(si.on_wait) if si is not None else []
                    if len(waits) > max_waits:
                        head, rest = waits[:max_waits], waits[max_waits:]
                        k = 0
                        while rest:
                            nop = mybir.InstNoOp(name=f"{inst.name}-w{k}", ins=[], outs=[])
                            nop.engine = inst.engine
                            nop.sync_info = mybir.SyncInfo(on_wait=rest[:max_waits],
                                                           on_update=[])
                            out.append(nop)
                            rest = rest[max_waits:]
                            k += 1
                        inst.sync_info = mybir.SyncInfo(on_wait=head,
                                                        on_update=list(si.on_update))
                        n_split += 1
                    out.append(inst)
                bb.instructions = out
        return n_split
    f32 = mybir.dt.float32
    AT = mybir.ActivationFunctionType
    OP = mybir.AluOpType
    nc = bass.Bass()

    CN = build_consts()
    dt = {k: nc.dram_tensor(f"c_{k}", list(v.shape), f32, kind="ExternalInput")
          for k, v in CN.items()}
    din = {}
    npair = n_batch * n_event
    shapes = {'noise': [npair, P, Q1], 'ginv': [npair, P, Q1],
              'p3': [npair, P, 3], 'uv': [npair, 2, 642],
              'res3': [npair, 86, 3]}
    for k, sshape in shapes.items():
        din[k] = nc.dram_tensor(k, sshape, f32, kind="ExternalInput")
    out_d = nc.dram_tensor("out", [n_batch, P, Q1], f32, kind="ExternalOutput")
    dbg_d = nc.dram_tensor("dbg", [6, P, Q2], f32, kind="ExternalOutput")

    with TileContextSplitDrain(nc) as tc:
        with tc.tile_pool(name="const", bufs=1) as cp, \
             tc.tile_pool(name="work", bufs=1) as wp, \
             tc.tile_pool(name="acc", bufs=1) as accp, \
             tc.tile_pool(name="pre", bufs=1) as prep, \
             tc.tile_pool(name="ps", bufs=1, space="PSUM") as pp, \
             tc.tile_pool(name="pst", bufs=2, space="PSUM") as pt_pool:
            ct = {}
            for k, v in CN.items():
                if v.ndim == 3:   # chunked: tile [rows, nch*cols]
                    nch, rows, cols = v.shape
                    t = cp.tile([rows, nch * cols], f32, name=f"c_{k}", tag=f"c_{k}")
                    for i in range(nch):
                        nc.sync.dma_start(t[:, i * cols:(i + 1) * cols], dt[k][i, :, :])
                else:
                    t = cp.tile(list(v.shape), f32, tag=f"c_{k}")
                    nc.sync.dma_start(t[:], dt[k][:])
                ct[k] = t

            def chunk_ap(name, i, cols):
                return ct[name][:, i * cols:(i + 1) * cols]

            def dbg_tap(stage, *aps):
                if DEBUG == stage:
                    for i, ap in enumerate(aps):
                        pp_, ff_ = ap.partition_size(), ap.free_size()
                        nc.sync.dma_start(dbg_d[i, 0:pp_, 0:ff_], ap)

            sig_tot = accp.tile([P, n_batch * Q1], f32, name="sigtot", tag="sigtot")

            def cmul(out_r, out_i, ar, ai, br, bi, tag, fdim):
                t1 = wp.tile([P, fdim], f32, name=tag + "1", tag=tag + "1")
                t2 = wp.tile([P, fdim], f32, name=tag + "2", tag=tag + "2")
                nc.vector.tensor_mul(t1[:], ar, br)
                nc.vector.tensor_mul(t2[:], ai, bi)
                nc.vector.tensor_sub(out_r, t1[:], t2[:])
                nc.vector.tensor_mul(t1[:], ar, bi)
                nc.vector.tensor_mul(t2[:], ai, br)
                nc.vector.tensor_add(out_i, t1[:], t2[:])

            def tr(out_psum, in_sbuf, neg=False, ksize=P, iname=None):
                if iname is None:
                    iname = 'nident' if neg else 'ident'
                elif neg:
                    iname = 'n' + iname
                nc.tensor.transpose(out_psum, in_sbuf, ct[iname][0:ksize, 0:ksize])

            def fft_stage2(btr, bti, nti, wr_name, wi_name, nch, cols, out_r, out_i=None):
                """X = (btr + i bti)^T-chunks @ (Wr + i Wi); nti = -bti."""
                for ch in range(nch):
                    cs = slice(ch * P, (ch + 1) * P)
                    nc.tensor.matmul(out_r, btr[:, cs], chunk_ap(wr_name, ch, cols),
                                     start=(ch == 0), stop=False)
                    if out_i is not None:
                        nc.tensor.matmul(out_i, btr[:, cs], chunk_ap(wi_name, ch, cols),
                                         start=(ch == 0), stop=False)
                for ch in range(nch):
                    cs = slice(ch * P, (ch + 1) * P)
                    nc.tensor.matmul(out_r, nti[:, cs], chunk_ap(wi_name, ch, cols),
                                     start=False, stop=(ch == nch - 1))
                    if out_i is not None:
                        nc.tensor.matmul(out_i, bti[:, cs], chunk_ap(wr_name, ch, cols),
                                         start=False, stop=(ch == nch - 1))

            def transpose_to_chunks(src_r, src_i, nch, tag):
                """[128, nch*128] -> transposed chunks [n2c, k1] side by side."""
                otr = wp.tile([P, nch * P], f32, name=tag + "r", tag=tag + "r")
                oti = wp.tile([P, nch * P], f32, name=tag + "i", tag=tag + "i")
                for ch in range(nch):
                    cs = slice(ch * P, (ch + 1) * P)
                    tp = pt_pool.tile([P, P], f32, name="T", tag="T")
                    tr(tp[:], src_r[:, cs])
                    nc.scalar.copy(otr[:, cs], tp[:])
                    tp2 = pt_pool.tile([P, P], f32, name="T", tag="T")
                    tr(tp2[:], src_i[:, cs])
                    nc.scalar.copy(oti[:, cs], tp2[:])
                nti = wp.tile([P, nch * P], f32, name=tag + "n", tag=tag + "n")
                nc.scalar.mul(nti[:], oti[:], -1.0)
                return otr, oti, nti

            if True:
                nc.vector.memset(sig_tot[:], 0.0)
                with tc.For_i(0, npair, 1) as ev:
                    xg = wp.tile([P, Q1], f32, name="xg", tag="xg")
                    nc.sync.dma_start(xg[:], din['noise'][ev, :, :])
                    xg = xg[:]

                    # ---------- forward FFT_N ----------
                    ps_a = pp.tile([P, Q2], f32, name="A", tag="A")
                    ps_b = pp.tile([P, Q2], f32, name="B", tag="B")
                    nc.tensor.matmul(ps_a[:, 0:Q1], ct['wpf_r'][:], xg, start=True, stop=True)
                    nc.tensor.matmul(ps_b[:, 0:Q1], ct['wpf_i'][:], xg, start=True, stop=True)
                    bpr = wp.tile([P, Q1], f32, name="bpr", tag="bpr")
                    bpi = wp.tile([P, Q1], f32, name="bpi", tag="bpi")
                    cmul(bpr[:], bpi[:], ps_a[:, 0:Q1], ps_b[:, 0:Q1],
                         ct['twf_r'][:], ct['twf_i'][:], "tA", Q1)
                    btr, bti, nti = transpose_to_chunks(bpr, bpi, 2, "bt")
                    ps_c = pp.tile([P, Q2], f32, name="C", tag="A")
                    ps_d = pp.tile([P, Q2], f32, name="D", tag="B")
                    fft_stage2(btr, bti, nti, 'wqf_r', 'wqf_i', 2, Q1,
                               ps_c[:, 0:Q1], ps_d[:, 0:Q1])
                    spr = wp.tile([P, Q1], f32, name="spr", tag="spr")
                    spi = wp.tile([P, Q1], f32, name="spi", tag="spi")
                    nc.scalar.copy(spr[:], ps_c[:, 0:Q1])
                    nc.scalar.copy(spi[:], ps_d[:, 0:Q1])
                    dbg_tap('sp', spr[:], spi[:])

                    # ---------- shape mult + inverse layout ----------
                    ginv = wp.tile([P, Q1], f32, name="ginv", tag="ginv")
                    nc.sync.dma_start(ginv[:], din['ginv'][ev, :, :])
                    ginv = ginv[:]
                    inv_r = wp.tile([P, Q1], f32, name="inv_r", tag="inv_r")
                    inv_i = wp.tile([P, Q1], f32, name="inv_i", tag="inv_i")
                    for src, dst in [(spr, inv_r), (spi, inv_i)]:
                        for half in range(2):
                            tp = pt_pool.tile([P, P], f32, name="T", tag="T")
                            tr(tp[:], src[:, half::2])
                            nc.vector.tensor_mul(dst[:, half * P:(half + 1) * P],
                                                 tp[:], ginv[:, half * P:(half + 1) * P])

                    dbg_tap('inv', inv_r[:], inv_i[:])
                    # ---------- inverse FFT_N -> band_noise (y-grid) ----------
                    ps_a = pp.tile([P, Q2], f32, name="A", tag="A")
                    ps_b = pp.tile([P, Q2], f32, name="B", tag="B")
                    nii = wp.tile([P, Q1], f32, name="nii", tag="nii")
                    nc.scalar.mul(nii[:], inv_i[:], -1.0)
                    nc.tensor.matmul(ps_a[:, 0:Q1], ct['wpi_r'][:], inv_r[:], start=True, stop=False)
                    nc.tensor.matmul(ps_a[:, 0:Q1], ct['wpi_i'][:], nii[:], start=False, stop=True)
                    nc.tensor.matmul(ps_b[:, 0:Q1], ct['wpi_i'][:], inv_r[:], start=True, stop=False)
                    nc.tensor.matmul(ps_b[:, 0:Q1], ct['wpi_r'][:], inv_i[:], start=False, stop=True)
                    cpr = wp.tile([P, Q1], f32, name="bpr", tag="bpr")
                    cpi = wp.tile([P, Q1], f32, name="bpi", tag="bpi")
                    cmul(cpr[:], cpi[:], ps_a[:, 0:Q1], ps_b[:, 0:Q1],
                         ct['twi_r'][:], ct['twi_i'][:], "tA", Q1)
                    ctr, cti, ncti = transpose_to_chunks(cpr, cpi, 2, "bt")
                    ps_c = pp.tile([P, Q2], f32, name="C", tag="A")
                    fft_stage2(ctr, cti, ncti, 'wqi_r', 'wqi_i', 2, Q1, ps_c[:, 0:Q1])

                    # ---------- probs * band_noise -> a_y ----------
                    p3t = wp.tile([P, 3], f32, name="p3t", tag="p3t")
                    nc.sync.dma_start(p3t[:], din['p3'][ev, :, :])
                    probs = wp.tile([P, Q1], f32, name="probs", tag="probs")
                    nc.scalar.activation(probs[:], ct['tsq'][:], AT.Exp,
                                         bias=p3t[:, 1:2], scale=p3t[:, 0:1])
                    a_y = wp.tile([P, Q1], f32, name="a_y", tag="a_y")
                    nc.vector.tensor_mul(a_y[:], probs[:], ps_c[:, 0:Q1])
                    dbg_tap('ay', a_y[:], probs[:])

                    # ---------- remap a_y -> a2 [64, 512] ----------
                    a2 = wp.tile([64, Q2], f32, name="a2", tag="a2")
                    for q in range(4):
                        tp = pt_pool.tile([P, P], f32, name="T", tag="T")
                        tr(tp[0:64, :], a_y[:, q::4])
                        nc.scalar.copy(a2[:, q * P:(q + 1) * P], tp[0:64, :])

                    dbg_tap('a2', a2[:])
                    # ---------- forward FFT_2N ----------
                    ps_a = pp.tile([P, Q2], f32, name="A", tag="A")
                    ps_b = pp.tile([P, Q2], f32, name="B", tag="B")
                    nc.tensor.matmul(ps_a[:], ct['wpf2_r'][0:64, :], a2[:], start=True, stop=True)
                    nc.tensor.matmul(ps_b[:], ct['wpf2_i'][0:64, :], a2[:], start=True, stop=True)
                    dpr = wp.tile([P, Q2], f32, name="dpr", tag="dpr")
                    dpi = wp.tile([P, Q2], f32, name="dpi", tag="dpi")
                    cmul(dpr[:], dpi[:], ps_a[:], ps_b[:],
                         ct['twf2_r'][:], ct['twf2_i'][:], "tB", Q2)
                    dtr, dti, ndti = transpose_to_chunks(dpr, dpi, 4, "dt")
                    ps_c = pp.tile([P, Q2], f32, name="C", tag="A")
                    ps_d = pp.tile([P, Q2], f32, name="D", tag="B")
                    fft_stage2(dtr, dti, ndti, 'wqf2_r', 'wqf2_i', 4, NCO,
                               ps_c[:, 0:NCO], ps_d[:, 0:NCO])
                    s2r = wp.tile([P, NCO], f32, name="s2r", tag="s2r")
                    s2i = wp.tile([P, NCO], f32, name="s2i", tag="s2i")
                    nc.scalar.copy(s2r[:], ps_c[:, 0:NCO])
                    nc.scalar.copy(s2i[:], ps_d[:, 0:NCO])
                    dbg_tap('s2', s2r[:], s2i[:])

                    # ---------- phase multiply ----------
                    uvt = wp.tile([2, 642], f32, name="uvt", tag="uvt")
                    nc.sync.dma_start(uvt[:], din['uv'][ev, :, :])
                    ps_a = pp.tile([P, Q2], f32, name="A", tag="A")
                    ps_b = pp.tile([P, Q2], f32, name="B", tag="B")
                    nc.tensor.matmul(ps_a[:, 0:NCO], uvt[:, 0:P], uvt[:, P:P + NCO],
                                     start=True, stop=True)
                    nc.tensor.matmul(ps_b[:, 0:NCO], uvt[:, 0:P], uvt[:, P + NCO:642],
                                     start=True, stop=True)
                    y_r = wp.tile([P, NCO], f32, name="y_r", tag="y_r")
                    y_i = wp.tile([P, NCO], f32, name="y_i", tag="y_i")
                    cmul(y_r[:], y_i[:], s2r[:], s2i[:],
                         ps_a[:, 0:NCO], ps_b[:, 0:NCO], "tC", NCO)
                    dbg_tap('y', y_r[:], y_i[:])

                    # ---------- hermitian extension: lo rows 0..63, hi rows 64..127 ----------
                    inv2_r = wp.tile([64, Q2], f32, name="inv2_r", tag="inv2_r")
                    inv2_i = wp.tile([64, Q2], f32, name="inv2_i", tag="inv2_i")
                    inv2h_r = wp.tile([64, Q2], f32, name="inv2h_r", tag="inv2h_r")
                    inv2h_i = wp.tile([64, Q2], f32, name="inv2h_i", tag="inv2h_i")
                    yrev_r = wp.tile([P, NCO], f32, name="yrev_r", tag="yrev_r")
                    yrev_i = wp.tile([P, NCO], f32, name="yrev_i", tag="yrev_i")
                    nc.vector.tensor_copy(yrev_r[:], y_r[:, 256::-1])
                    nc.vector.tensor_copy(yrev_i[:], y_i[:, 256::-1])
                    # region A rows 0..63: Inv2[s1, 128q+lo] = Y[lo, 4 s1 + q]
                    for q in range(4):
                        for yy, dst in [(y_r, inv2_r), (y_i, inv2_i)]:
                            tp = pt_pool.tile([P, P], f32, name="T", tag="T")
                            tr(tp[0:64, :], yy[:, q::4][:, 0:64])
                            nc.scalar.copy(dst[:, q * P:(q + 1) * P], tp[0:64, :])
                    # hi rows (global 64+a): col 0: conj(Y[0, 256-4a]) = conj(yrev[0,4a])
                    for yy, dst, ng in [(yrev_r, inv2h_r, False), (yrev_i, inv2h_i, True)]:
                        tp = pt_pool.tile([P, P], f32, name="T", tag="T")
                        tr(tp[0:64, 0:1], yy[0:1, 0:256:4], ksize=1)
                        if ng:
                            nc.scalar.mul(dst[:, 0:1], tp[0:64, 0:1], -1.0)
                        else:
                            nc.scalar.copy(dst[:, 0:1], tp[0:64, 0:1])
                    # ...except global row 64 col 0 = direct Y[0, 256]
                    nc.scalar.copy(inv2h_r[0:1, 0:1], y_r[0:1, 256:257])
                    nc.scalar.copy(inv2h_i[0:1, 0:1], y_i[0:1, 256:257])
                    # region B: hi[a, 128*blk+mu] = conj(Y[128-mu, 252+qp-4a]) via J-transpose
                    for qp in range(4):
                        blk = 3 - qp
                        for yy, dst, ng in [(yrev_r, inv2h_r, False), (yrev_i, inv2h_i, True)]:
                            tp = pt_pool.tile([P, P], f32, name="T", tag="T")
                            tr(tp[0:64, :], yy[:, 4 - qp::4][:, 0:64], iname='jrev')
                            if ng:
                                nc.scalar.mul(dst[:, blk * P + 1:blk * P + 128],
                                              tp[0:64, 0:127], -1.0)
                            else:
                                nc.scalar.copy(dst[:, blk * P + 1:blk * P + 128],
                                               tp[0:64, 0:127])
                        if qp >= 1:
                            # hi[a, (4-qp)*128] = conj(Y[0, 252+qp-4a]) = conj(yrev[0, 4-qp+4a])
                            for yy, dst, ng in [(yrev_r, inv2h_r, False), (yrev_i, inv2h_i, True)]:
                                tp = pt_pool.tile([P, P], f32, name="T", tag="T")
                                tr(tp[0:64, 0:1], yy[0:1, 4 - qp::4][:, 0:64], ksize=1)
                                if ng:
                                    nc.scalar.mul(dst[:, (4 - qp) * P:(4 - qp) * P + 1],
                                                  tp[0:64, 0:1], -1.0)
                                else:
                                    nc.scalar.copy(dst[:, (4 - qp) * P:(4 - qp) * P + 1],
                                                   tp[0:64, 0:1])

                    dbg_tap('inv2', inv2_r[:], inv2_i[:], inv2h_r[:], inv2h_i[:])
                    # ---------- inverse FFT_2N -> atoms (y-grid, real) ----------
                    ps_a = pp.tile([P, Q2], f32, name="A", tag="A")
                    ps_b = pp.tile([P, Q2], f32, name="B", tag="B")
                    ni2 = wp.tile([64, Q2], f32, name="ni2", tag="ni2")
                    ni2h = wp.tile([64, Q2], f32, name="ni2h", tag="ni2h")
                    nc.scalar.mul(ni2[:], inv2_i[:], -1.0)
                    nc.scalar.mul(ni2h[:], inv2h_i[:], -1.0)
                    nc.tensor.matmul(ps_a[:], ct['wpi2_r'][0:64, :], inv2_r[:], start=True, stop=False)
                    nc.tensor.matmul(ps_a[:], ct['wpi2h_r'][:], inv2h_r[:], start=False, stop=False)
                    nc.tensor.matmul(ps_a[:], ct['wpi2_i'][0:64, :], ni2[:], start=False, stop=False)
                    nc.tensor.matmul(ps_a[:], ct['wpi2h_i'][:], ni2h[:], start=False, stop=True)
                    nc.tensor.matmul(ps_b[:], ct['wpi2_i'][0:64, :], inv2_r[:], start=True, stop=False)
                    nc.tensor.matmul(ps_b[:], ct['wpi2h_i'][:], inv2h_r[:], start=False, stop=False)
                    nc.tensor.matmul(ps_b[:], ct['wpi2_r'][0:64, :], inv2_i[:], start=False, stop=False)
                    nc.tensor.matmul(ps_b[:], ct['wpi2h_r'][:], inv2h_i[:], start=False, stop=True)
                    epr = wp.tile([P, Q2], f32, name="dpr", tag="dpr")
                    epi = wp.tile([P, Q2], f32, name="dpi", tag="dpi")
                    cmul(epr[:], epi[:], ps_a[:], ps_b[:],
                         ct['twi2_r'][:], ct['twi2_i'][:], "tB", Q2)
                    etr, eti, neti = transpose_to_chunks(epr, epi, 4, "dt")
                    ps_c = pp.tile([P, Q2], f32, name="C", tag="A")
                    fft_stage2(etr, eti, neti, 'wqi2_r', 'wqi2_i', 4, Q1, ps_c[:, 0:Q1])

                    # ---------- atoms * amps -> SBUF (padded +2 cols) ----------
                    atoms = wp.tile([P, Q1 + 2], f32, name="atoms", tag="atoms")
                    nc.vector.memset(atoms[:, Q1:Q1 + 2], 0.0)
                    nc.scalar.activation(atoms[:, 0:Q1], ps_c[:, 0:Q1], AT.Copy,
                                         scale=p3t[:, 2:3])

                    dbg_tap('atoms', atoms[:])
                    # ---------- frame DFT + scan + inverse frame DFT ----------
                    rest = wp.tile([86, 3], f32, name="rest", tag="rest")
                    nc.sync.dma_start(rest[:], din['res3'][ev, :, :])
                    fin_r = [wp.tile([hi - lo, NF], f32, name=f"finr{i}", tag=f"finr{i}") for i, (lo, hi) in enumerate(CT)]
                    fin_i = [wp.tile([hi - lo, NF], f32, name=f"fini{i}", tag=f"fini{i}") for i, (lo, hi) in enumerate(CT)]
                    for i, (lo, hi) in enumerate(CT):
                        n_c = hi - lo
                        sp_r = pt_pool.tile([P, NF], f32, name="S", tag="S")
                        sp_i = pt_pool.tile([P, NF], f32, name="S", tag="S")
                        for u in range(4):
                            rhs = atoms[:, u:min(u + 256, 258):2]
                            nc.tensor.matmul(sp_r[0:n_c, :],
                                             chunk_ap('hd_r', u, NCO)[:, lo:hi], rhs,
                                             start=(u == 0), stop=(u == 3))
                            nc.tensor.matmul(sp_i[0:n_c, :],
                                             chunk_ap('hd_i', u, NCO)[:, lo:hi], rhs,
                                             start=(u == 0), stop=(u == 3))
                        rb = wp.tile([P, NF], f32, name="rb", tag="rb")
                        nc.scalar.activation(rb[0:n_c, :], ct['tsq'][0:n_c, 0:NF],
                                             AT.Identity, bias=rest[0:n_c, i:i + 1], scale=0.0)
                        nc.vector.tensor_tensor_scan(fin_r[i][:], rb[0:n_c, :], sp_r[0:n_c, :],
                                                     initial=sp_r[0:n_c, 0:1],
                                                     op0=mybir.AluOpType.mult,
                                                     op1=mybir.AluOpType.add)
                        nc.vector.tensor_tensor_scan(fin_i[i][:], rb[0:n_c, :], sp_i[0:n_c, :],
                                                     initial=sp_i[0:n_c, 0:1],
                                                     op0=mybir.AluOpType.mult,
                                                     op1=mybir.AluOpType.add)

                    sg = wp.tile([P, Q1], f32, name="sg", tag="sg")
                    for u in range(4):
                        of = pt_pool.tile([P, NF], f32, name="O", tag="O")
                        ws = slice(u * P, (u + 1) * P)
                        for i, (lo, hi) in enumerate(CT):
                            nc.tensor.matmul(of[:], ct['e_r'][0:hi - lo, ws] if False else
                                             chunk_ap('e_r', i, Q2)[0:hi - lo, ws],
                                             fin_r[i][:], start=(i == 0), stop=False)
                            nc.tensor.matmul(of[:], chunk_ap('e_i', i, Q2)[0:hi - lo, ws],
                                             fin_i[i][:], start=False, stop=(i == 2))
                        # ---------- OLA ----------
                        if u < 2:
                            nc.scalar.copy(sg[:, u::2], of[:])
                        else:
                            nc.vector.tensor_add(sg[:, u::2], sg[:, u::2], of[:, 0:127])
                    dbg_tap('sg', sg[:])
                    dbg_tap('fin', *[f[:] for f in fin_r[:3]], *[f[:] for f in fin_i[:3]])
                    sslice = sig_tot[:, bass.ds((ev // n_event) * Q1, Q1)]
                    nc.vector.tensor_add(sslice, sslice, sg[:])

            for b in range(n_batch):
                # ---------- max_norm ----------
                sb_ = sig_tot[:, b * Q1:(b + 1) * Q1]
                mx = wp.tile([P, 1], f32, name="mx", tag="mx")
                nc.vector.tensor_reduce(mx[:], sb_, axis=mybir.AxisListType.X,
                                        op=mybir.AluOpType.max, apply_absolute_value=True)
                tpm = pt_pool.tile([P, P], f32, name="T", tag="T")
                tr(tpm[0:1, :], mx[:])
                mxs = wp.tile([1, P], f32, name="mxs", tag="mxs")
                nc.scalar.copy(mxs[:], tpm[0:1, :])
                m11 = wp.tile([1, 1], f32, name="m11", tag="m11")
                nc.vector.tensor_reduce(m11[:], mxs[:], axis=mybir.AxisListType.X,
                                        op=mybir.AluOpType.max)
                bc = pt_pool.tile([P, P], f32, name="T", tag="T")
                nc.tensor.matmul(bc[:, 0:1], ct['ones1'][:], m11[:], start=True, stop=True)
                bcs = wp.tile([P, 1], f32, name="bcs", tag="bcs")
                nc.vector.tensor_scalar_add(bcs[:], bc[:, 0:1], 1e-8)
                rcp = wp.tile([P, 1], f32, name="rcp", tag="rcp")
                nc.vector.reciprocal(rcp[:], bcs[:])
                outt = wp.tile([P, Q1], f32, name="outt", tag="outt")
                nc.scalar.activation(outt[:], sb_, AT.Copy, scale=rcp[:])
                nc.sync.dma_start(out_d[b, :, :], outt[:])

    split_excess_waits(nc)
    return nc, CN


def kernel(x, noise):
    from concourse.bass_utils import run_bass_kernel_spmd
    x = np.asarray(x, dtype=np.float32)
    noise = np.asarray(noise, dtype=np.float32)
    B, E = x.shape[:2]
    n_cores = 8
    nb = B // n_cores
    nc, CN = build_program(nb, E)
    pd = build_pair_data(x, noise)
    in_maps = []
    for c in range(n_cores):
        m = {f"c_{k}": v for k, v in CN.items()}
        sl = slice(c * nb, (c + 1) * nb)
        m['noise'] = np.ascontiguousarray(
            pd['noise'][sl].reshape(nb * E, P, Q1))
        for k in ['ginv', 'p3', 'uv', 'res3']:
            v = pd[k][sl]
            m[k] = np.ascontiguousarray(v.reshape(nb * E, *v.shape[2:]))
        in_maps.append(m)
    res = run_bass_kernel_spmd(nc, in_maps, core_ids=list(range(n_cores)))
    global LAST_RESULT
    LAST_RESULT = res
    out = np.zeros((B, 1, NS), dtype=np.float32)
    for c in range(n_cores):
        o = res.results[c]['out']
        for bb in range(nb):
            out[c * nb + bb, 0, :] = o[bb].T.reshape(-1)
    return out
